# revision 33
# baseline (speedup 1.0000x reference)
"""Trainium2 Bass kernel for nn_Loss_20993800143146 (loss_fn).

Computes, over 8 NeuronCores (data-parallel over batch / bh):
    mel_loss  = mean(|mels_pred * mask - mels_target|)           (mean over full tensor)
    stop_loss = sum(-5 * log(stop_pred[b, last_idx_b])) / mask.sum()
    dc        = sum(alignments * band[s,t] * bmask[b]) / (H * lengths.sum() * N)
    out       = mel_loss + stop_loss - 1e-4 * dc

Key algebraic facts:
  * band[s,t] = (s >= clip(5t-50,0,160)) & (s < clip(5t+50,0,160)) is zero for
    t >= 42, and within t < 42 only 2975 of the 6720 (s,t) cells are nonzero.
    The host packs EXACTLY the banded elements densely (zero-padded to a
    rectangle), so the device just sums them - no band weights needed.
  * The mel mask multiplies mels_pred only, so it is folded into the host-side
    packing of the pred tile (masked positions packed as 0), leaving a plain
    sum(|p - t|) on device.

Sharding: batch dim (16 -> 2 per core) for mask/stop/mels, bh dim (64 -> 8 per
core) for alignments. Each core reduces its shard to 6 partial f32 stats; the
host sums the 8x6 partials and applies the constant-denominator arithmetic.

Heavy data in bf16 (mels) / fp8-e4m3 (alignments); rel-err budget is 2e-2,
measured error stays ~1e-4.

Input DMA is phased through ONE dram tensor on the sync path (HW queues
process descriptors in trigger order at full aggregate bandwidth):
  phase S2 (small, first): 128x128 f32 identity (PE transpose weights), banded
     alignments fp8, stop/mask/iota f16 in the split-per-b layout (b = p//64,
     t = 13*(p%64)+j), per-partition b length f32, ones f32 (matmul rhs),
     length prefill f32 - unblocks the stop and dc chains early;
  phase S1: mels_pred*mask and mels_target bf16 (13 rows of 80 per partition).

Stop-term selection per b (v1-proven PE pattern, no GpSimd): per-partition
masked-iota max mxp and masked ln-candidate cp, PE-transpose both to rows on
partition 0, per-b max over the 64-wide halves, then is_equal-select.

Stats cols (own memset tile): 0=dc_w, 1=melA, 2=sel_b0 (part 0), 3=mask_cnt,
4=len, 5=sel_b1 (part 0). A PE matmul ones.T @ stats collapses [128,6] ->
[1,6] in PSUM; one-descriptor DMA out.
"""

import numpy as np
import ml_dtypes

# Problem constants (hardcoded per contract; kernel.py must be self-contained).
H = 4
B = 16
T = 800
NMEL = 80
S = 160
N = 3
BW = 50
K = T // S  # 5
TC = 42  # band[:, t] == 0 for all t >= TC
NCORES = 8

MEL_ROWS = 2 * T            # 1600 (b,t) rows per core
MEL_PAD_ROWS = 1664         # pad to 128 * 13
MG = 13                     # 80-col groups per partition / stop t's per part
MEL_F = MG * NMEL           # 1040 mel elements per partition per tensor
ALN_PER_PLANE = 2975        # nonzero band cells per (n, bh) plane
ALN_PER_PART = 560          # 16*560 >= 3*2975, zero padded
ALN_HALF = ALN_PER_PART // 2  # 280

# dAll byte offsets
# phase S0 (scalar path, first): stop-term sidecar, 128 B
O_STOP = 0                    # 13 f16
O_MASK = O_STOP + 2 * MG      # 26
O_IOTA = O_MASK + 2 * MG      # 52
O_LEN = 80                    # f32
O_LPRE = 84                   # f32: lengths at partitions 0/1, 0 elsewhere
O_S0END = 128
# phases S1a / S1b (sync path): mel halves
MH = MEL_F // 2               # 520 els per mel half
O_P1 = O_S0END                # mel pred half1, 520 bf16
O_T1 = O_P1 + 2 * MH          # 1168
O_P2 = O_T1 + 2 * MH          # 2208
O_T2 = O_P2 + 2 * MH          # 3248
O_S1END = O_T2 + 2 * MH       # 4288
# phase S2b (scalar path, lands between the mel halves): id + alignments
O_ID = O_S1END                # 128 f32 identity row
O_ALN = O_ID + 512            # 560 fp8
W_ALL = O_ALN + 560           # 5360

_CACHE = {}


def _band_bool():
    tr = np.arange(TC)
    mn = np.clip(K * tr - BW, 0, S)
    mx = np.clip(K * tr + BW, 0, S)
    rows = np.arange(S)
    return (rows[:, None] >= mn[None, :]) & (rows[:, None] < mx[None, :])


def _build_bass():
    import concourse.bacc as bacc
    import concourse.tile as tile
    import concourse.mybir as mybir
    from contextlib import ExitStack

    f32 = mybir.dt.float32
    f16 = mybir.dt.float16
    bf16 = mybir.dt.bfloat16
    fp8 = mybir.dt.float8e4
    u8 = mybir.dt.uint8
    Alu = mybir.AluOpType
    Act = mybir.ActivationFunctionType
    Ax = mybir.AxisListType

    nc = bacc.Bacc("TRN2", target_bir_lowering=False, debug=False,
                   num_devices=NCORES)

    dAll = nc.dram_tensor("dAll", [128, W_ALL], u8, kind="ExternalInput").ap()
    out = nc.dram_tensor("out", [128, 8], f32, kind="ExternalOutput").ap()

    with tile.TileContext(nc) as tc:
        with ExitStack() as ctx:
            pool = ctx.enter_context(tc.tile_pool(name="main", bufs=1))
            ppool = ctx.enter_context(tc.tile_pool(name="ps", bufs=1,
                                                   space="PSUM"))

            td_t = pool.tile([128, W_ALL], u8, tag="td")

            # Phased DMA triggers, all on the sync sequencer; the shared
            # queue set executes transfers in doorbell order: S0 (tiny stop
            # sidecar), S1 (both mel halves), S2b (id + aln).
            nc.sync.dma_start(td_t[:, 0:O_S0END], dAll[:, 0:O_S0END])
            nc.sync.dma_start(td_t[:, O_P1:O_S1END], dAll[:, O_P1:O_S1END])
            nc.sync.dma_start(td_t[:, O_S1END:W_ALL], dAll[:, O_S1END:W_ALL])

            id_v = td_t[:, O_ID:O_ALN].bitcast(f32)           # [128, 128]
            aln_v = td_t[:, O_ALN:W_ALL].bitcast(fp8)         # [128, 560]
            stop_v = td_t[:, O_STOP:O_MASK].bitcast(f16)      # [128, 13]
            mask_v = td_t[:, O_MASK:O_IOTA].bitcast(f16)
            iota_v = td_t[:, O_IOTA:O_IOTA + 2 * MG].bitcast(f16)
            lenf_v = td_t[:, O_LEN:O_LEN + 4].bitcast(f32)
            lpre_v = td_t[:, O_LPRE:O_LPRE + 4].bitcast(f32)  # [128, 1]
            p1_v = td_t[:, O_P1:O_T1].bitcast(bf16)           # [128, 520]
            t1_v = td_t[:, O_T1:O_P2].bitcast(bf16)
            p2_v = td_t[:, O_P2:O_T2].bitcast(bf16)
            t2_v = td_t[:, O_T2:O_S1END].bitcast(bf16)

            st_t = pool.tile([128, 8], f32, tag="st")
            stats = st_t[:]

            # ---- ACT: Ln for the stop term ----
            lnp_t = pool.tile([128, MG], f32, tag="lnp")
            nc.scalar.activation(lnp_t[:], stop_v, Act.Ln)

            # ---- DVE stop front + dc term (phase S2 data only) ----
            tl_t = pool.tile([128, MG], f32, tag="tl")
            nc.vector.scalar_tensor_tensor(
                tl_t[:], iota_v, 1.0, mask_v, op0=Alu.bypass, op1=Alu.mult)
            mxp_t = pool.tile([128, 1], f32, tag="mxp")
            nc.vector.tensor_reduce(mxp_t[:], tl_t[:], axis=Ax.X, op=Alu.max)
            nc.vector.tensor_reduce(stats[:, 3:4], mask_v, axis=Ax.X,
                                    op=Alu.add)
            nc.vector.tensor_copy(stats[:, 4:5], lpre_v)
            asum_t = pool.tile([128, ALN_HALF], bf16, tag="asum")
            dcs_t = pool.tile([128, 1], f32, tag="dcs")
            nc.vector.scalar_tensor_tensor(
                asum_t[:], aln_v[:, 0:ALN_HALF], 1.0, aln_v[:, ALN_HALF:],
                op0=Alu.bypass, op1=Alu.add, accum_out=dcs_t[:])
            nc.vector.scalar_tensor_tensor(
                stats[:, 0:1], lenf_v, float(T), dcs_t[:],
                op0=Alu.is_le, op1=Alu.mult)
            eq_t = pool.tile([128, MG], f32, tag="eq")
            cp_t = pool.tile([128, 1], f32, tag="cp")
            nc.vector.scalar_tensor_tensor(
                eq_t[:], tl_t[:], mxp_t[:, 0:1], lnp_t[:],
                op0=Alu.is_equal, op1=Alu.mult, accum_out=cp_t[:])

            # ---- stop tail: PE transposes, per-b max, select ----
            psA = ppool.tile([1, 128], f32, tag="psA")
            nc.tensor.matmul(psA[:], lhsT=mxp_t[:], rhs=id_v,
                             start=True, stop=True)
            psB = ppool.tile([1, 128], f32, tag="psB")
            nc.tensor.matmul(psB[:], lhsT=cp_t[:], rhs=id_v,
                             start=True, stop=True)
            sbA_t = pool.tile([1, 128], f32, tag="sbA")
            nc.vector.tensor_copy(sbA_t[:], psA[:])
            mb_t = pool.tile([1, 2], f32, tag="mb")
            nc.vector.tensor_reduce(
                mb_t[:], sbA_t[:].rearrange("p (b g) -> p b g", g=64),
                axis=Ax.X, op=Alu.max)
            e0_t = pool.tile([1, 64], f32, tag="e0")
            nc.vector.scalar_tensor_tensor(
                e0_t[:], sbA_t[0:1, 0:64], mb_t[:, 0:1], psB[0:1, 0:64],
                op0=Alu.is_equal, op1=Alu.mult, accum_out=stats[0:1, 2:3])
            e1_t = pool.tile([1, 64], f32, tag="e1")
            nc.vector.scalar_tensor_tensor(
                e1_t[:], sbA_t[0:1, 64:128], mb_t[:, 1:2], psB[0:1, 64:128],
                op0=Alu.is_equal, op1=Alu.mult, accum_out=stats[0:1, 5:6])

            # ---- mel term: halves pipelined behind phases S1a / S1b ----
            d_t = pool.tile([128, MEL_F], bf16, tag="d")
            nc.vector.tensor_sub(d_t[:, 0:MH], p1_v, t1_v)
            nc.vector.tensor_reduce(stats[:, 1:2], d_t[:, 0:MH], axis=Ax.X,
                                    op=Alu.add, apply_absolute_value=True)
            nc.vector.tensor_sub(d_t[:, MH:MEL_F], p2_v, t2_v)
            nc.vector.tensor_reduce(stats[:, 6:7], d_t[:, MH:MEL_F],
                                    axis=Ax.X, op=Alu.add,
                                    apply_absolute_value=True)

            # ---- stats go out raw; the host folds the 128 partitions ----
            nc.sync.dma_start(out, st_t[:])

    nc.compile()
    return nc


def _get_nc():
    if "nc" not in _CACHE:
        _CACHE["nc"] = _build_bass()
    return _CACHE["nc"]


def make_in_maps(lengths, mask, stop_pred, mels_pred, mels_target, alignments):
    """Shard full inputs into the 8 per-core input dicts."""
    lengths = np.ascontiguousarray(lengths, dtype=np.int32)
    maskf = np.ascontiguousarray(mask).astype(np.float32)
    stop_pred = np.ascontiguousarray(stop_pred, dtype=np.float32)
    mels_pred = np.ascontiguousarray(mels_pred, dtype=np.float32)
    mels_target = np.ascontiguousarray(mels_target, dtype=np.float32)
    alignments = np.ascontiguousarray(alignments, dtype=np.float32)

    bf = ml_dtypes.bfloat16
    f8 = ml_dtypes.float8_e4m3
    band = _band_bool()  # [S, TC]
    el = alignments[:, :, :, :TC][:, :, band]  # [N, B*H, 2975]

    def split13(row, pad):
        o = np.full((64 * MG,), pad, row.dtype)
        o[:T] = row
        return o.reshape(64, MG)

    iota13 = np.concatenate([split13(np.arange(1, T + 1, dtype=np.float16),
                                     np.float16(0))] * 2)  # [128, 13]
    ident = np.eye(128, dtype=np.float32)

    def pad_rows(x2d):
        padded = np.zeros((MEL_PAD_ROWS, NMEL), x2d.dtype)
        padded[:MEL_ROWS] = x2d
        return padded.reshape(128, MEL_F)

    in_maps = []
    for c in range(NCORES):
        bs = slice(2 * c, 2 * c + 2)
        mp = pad_rows((mels_pred[bs] * maskf[bs][..., None])
                      .reshape(MEL_ROWS, NMEL).astype(bf))
        mt = pad_rows(mels_target[bs].reshape(MEL_ROWS, NMEL).astype(bf))

        aln = np.zeros((8, 16 * ALN_PER_PART), f8)
        core_el = el[:, 8 * c:8 * c + 8]          # [3, 8, 2975]
        aln[:, :N * ALN_PER_PLANE] = \
            core_el.transpose(1, 0, 2).reshape(8, N * ALN_PER_PLANE).astype(f8)
        aln = aln.reshape(128, ALN_PER_PART)

        dAll = np.zeros((128, W_ALL), np.uint8)
        dAll[:, O_ID:O_ALN] = ident.view(np.uint8)
        dAll[:, O_ALN:W_ALL] = aln.view(np.uint8)
        st13 = np.concatenate(
            [split13(stop_pred[2 * c].astype(np.float16), np.float16(1.0)),
             split13(stop_pred[2 * c + 1].astype(np.float16), np.float16(1.0))])
        mk13 = np.concatenate(
            [split13(maskf[2 * c].astype(np.float16), np.float16(0)),
             split13(maskf[2 * c + 1].astype(np.float16), np.float16(0))])
        dAll[:, O_STOP:O_MASK] = st13.view(np.uint8)
        dAll[:, O_MASK:O_IOTA] = mk13.view(np.uint8)
        dAll[:, O_IOTA:O_IOTA + 2 * MG] = iota13.view(np.uint8)
        lenf = np.repeat(lengths[bs].astype(np.float32), 64)  # [128]
        dAll[:, O_LEN:O_LEN + 4] = lenf[:, None].view(np.uint8)
        lpre = np.zeros((128, 1), np.float32)
        lpre[0:2, 0] = lengths[bs]
        dAll[:, O_LPRE:O_LPRE + 4] = lpre.view(np.uint8)
        dAll[:, O_P1:O_T1] = mp[:, 0:MH].view(np.uint8)
        dAll[:, O_T1:O_P2] = np.ascontiguousarray(mt[:, 0:MH]).view(np.uint8)
        dAll[:, O_P2:O_T2] = np.ascontiguousarray(mp[:, MH:]).view(np.uint8)
        dAll[:, O_T2:O_S1END] = np.ascontiguousarray(mt[:, MH:]).view(np.uint8)

        in_maps.append({"dAll": dAll})
    return in_maps


def combine_partials(partials):
    """partials: list of 8 arrays [128,8] -> final scalar (0-d f32 ndarray).

    Cols 2/5 (stop selection) are only written on partition 0; the other
    partitions hold uninitialized SBUF and are ignored.
    """
    ps = np.stack([np.asarray(p, dtype=np.float64).reshape(128, 8)
                   for p in partials])
    tot = ps.sum(axis=(0, 1))
    p0 = ps[:, 0, :].sum(axis=0)
    dc_w, mask_cnt, len_sum = tot[0], tot[3], tot[4]
    melA = tot[1] + tot[6]
    sel_lnp = p0[2] + p0[5]
    mel_loss = melA / float(B * T * NMEL)
    stop_loss = -5.0 * sel_lnp / mask_cnt
    dc = dc_w / (H * len_sum * N)
    return np.array(np.float32(mel_loss + stop_loss - 1e-4 * dc))


def kernel(lengths, mask, stop_pred, mels_pred, mels_target, alignments):
    from concourse.bass_utils import run_bass_kernel_spmd

    nc = _get_nc()
    in_maps = make_in_maps(lengths, np.asarray(mask), stop_pred,
                           mels_pred, mels_target, alignments)
    res = run_bass_kernel_spmd(nc, in_maps, list(range(NCORES)))
    return combine_partials([r["out"] for r in res.results])


# revision 34
# speedup vs baseline: 1.0729x; 1.0729x over previous
"""Trainium2 Bass kernel for nn_Loss_20993800143146 (loss_fn).

Computes, over 8 NeuronCores (data-parallel over batch / bh):
    mel_loss  = mean(|mels_pred * mask - mels_target|)           (mean over full tensor)
    stop_loss = sum(-5 * log(stop_pred[b, last_idx_b])) / mask.sum()
    dc        = sum(alignments * band[s,t] * bmask[b]) / (H * lengths.sum() * N)
    out       = mel_loss + stop_loss - 1e-4 * dc

Key algebraic facts:
  * band[s,t] = (s >= clip(5t-50,0,160)) & (s < clip(5t+50,0,160)) is zero for
    t >= 42, and within t < 42 only 2975 of the 6720 (s,t) cells are nonzero.
    The host packs EXACTLY the banded elements densely (zero-padded to a
    rectangle), so the device just sums them - no band weights needed.
  * The mel mask multiplies mels_pred only, so it is folded into the host-side
    packing of the pred tile (masked positions packed as 0), leaving a plain
    sum(|p - t|) on device.

Sharding: batch dim (16 -> 2 per core) for mask/stop/mels, bh dim (64 -> 8 per
core) for alignments. Each core reduces its shard to a [128,8] stats tile; the
host folds partitions / cores and applies the constant-denominator arithmetic.

Heavy data in bf16 (mels) / fp8-e4m3 (alignments) / f16 (stop sidecar and
transpose identity); rel-err budget is 2e-2, measured error stays ~1e-4.

Input DMA is phased through ONE dram tensor on the sync path (the shared HW
queue set executes transfers in doorbell order at full aggregate bandwidth):
  phase S2 (first): f16 identity (PE transpose weights), banded alignments
     fp8, stop/mask/iota f16 in the split-per-b layout (b = p//64,
     t = 13*(p%64)+j), per-partition b length f32, length prefill f32 -
     unblocks the stop and dc chains while the mels stream;
  phases S1a / S1b: mel halves (pred*mask | target bf16 each), so the two
     subtract+|.|-reduce pairs pipeline behind the DMA.

Stop-term selection per b (PE pattern, no GpSimd): per-partition masked-iota
max mxp and masked ln-candidate cp (both f16), PE-transpose both to rows on
partition 0 via the identity, per-b max over the 64-wide halves, then one
is_equal-select per b.

Stats cols: 0=dc_w, 1=melA_h1, 2=sel_b0 (part 0 only), 3=mask_cnt, 4=len,
5=sel_b1 (part 0 only), 6=melA_h2, 7=unused. The [128,8] tile goes out raw;
the host ignores the unwritten lanes of cols 2/5/7.
"""

import numpy as np
import ml_dtypes

# Problem constants (hardcoded per contract; kernel.py must be self-contained).
H = 4
B = 16
T = 800
NMEL = 80
S = 160
N = 3
BW = 50
K = T // S  # 5
TC = 42  # band[:, t] == 0 for all t >= TC
NCORES = 8

MEL_ROWS = 2 * T            # 1600 (b,t) rows per core
MEL_PAD_ROWS = 1664         # pad to 128 * 13
MG = 13                     # 80-col groups per partition / stop t's per part
MEL_F = MG * NMEL           # 1040 mel elements per partition per tensor
MH = MEL_F // 2             # 520 els per mel half
ALN_PER_PLANE = 2975        # nonzero band cells per (n, bh) plane
ALN_PER_PART = 560          # 16*560 >= 3*2975, zero padded
ALN_HALF = ALN_PER_PART // 2  # 280

# dAll byte offsets -- phase S2
O_ID = 0                      # 128 f16 identity row (256 B)
O_ALN = 256                   # 560 fp8
O_STOP = O_ALN + 560          # 816, 13 f16
O_MASK = O_STOP + 2 * MG      # 842
O_IOTA = O_MASK + 2 * MG      # 868
O_LEN = 896                   # f32
O_LPRE = 900                  # f32: lengths at partitions 0/1, 0 elsewhere
O_S2END = 912
# phases S1a / S1b
O_P1 = O_S2END                # mel pred half1, 520 bf16
O_T1 = O_P1 + 2 * MH          # 1952
O_P2 = O_T1 + 2 * MH          # 2992
O_T2 = O_P2 + 2 * MH          # 4032
W_ALL = O_T2 + 2 * MH         # 5072

_CACHE = {}


def _band_bool():
    tr = np.arange(TC)
    mn = np.clip(K * tr - BW, 0, S)
    mx = np.clip(K * tr + BW, 0, S)
    rows = np.arange(S)
    return (rows[:, None] >= mn[None, :]) & (rows[:, None] < mx[None, :])


def _build_bass():
    import concourse.bacc as bacc
    import concourse.tile as tile
    import concourse.mybir as mybir
    from contextlib import ExitStack

    f32 = mybir.dt.float32
    f16 = mybir.dt.float16
    bf16 = mybir.dt.bfloat16
    fp8 = mybir.dt.float8e4
    u8 = mybir.dt.uint8
    Alu = mybir.AluOpType
    Act = mybir.ActivationFunctionType
    Ax = mybir.AxisListType

    nc = bacc.Bacc("TRN2", target_bir_lowering=False, debug=False,
                   num_devices=NCORES)

    dAll = nc.dram_tensor("dAll", [128, W_ALL], u8, kind="ExternalInput").ap()
    out = nc.dram_tensor("out", [128, 8], f32, kind="ExternalOutput").ap()

    with tile.TileContext(nc) as tc:
        with ExitStack() as ctx:
            pool = ctx.enter_context(tc.tile_pool(name="main", bufs=1))
            ppool = ctx.enter_context(tc.tile_pool(name="ps", bufs=1,
                                                   space="PSUM"))

            td_t = pool.tile([128, W_ALL], u8, tag="td")

            # Phased DMA triggers, all on the sync sequencer; the shared
            # queue set executes them in doorbell order.
            nc.sync.dma_start(td_t[:, 0:O_S2END], dAll[:, 0:O_S2END])
            nc.sync.dma_start(td_t[:, O_P1:O_P2], dAll[:, O_P1:O_P2])
            nc.sync.dma_start(td_t[:, O_P2:W_ALL], dAll[:, O_P2:W_ALL])

            id_v = td_t[:, O_ID:O_ALN].bitcast(f16)           # [128, 128]
            aln_v = td_t[:, O_ALN:O_STOP].bitcast(fp8)        # [128, 560]
            stop_v = td_t[:, O_STOP:O_MASK].bitcast(f16)      # [128, 13]
            mask_v = td_t[:, O_MASK:O_IOTA].bitcast(f16)
            iota_v = td_t[:, O_IOTA:O_IOTA + 2 * MG].bitcast(f16)
            lenf_v = td_t[:, O_LEN:O_LEN + 4].bitcast(f32)
            lpre_v = td_t[:, O_LPRE:O_LPRE + 4].bitcast(f32)  # [128, 1]
            p1_v = td_t[:, O_P1:O_T1].bitcast(bf16)           # [128, 520]
            t1_v = td_t[:, O_T1:O_P2].bitcast(bf16)
            p2_v = td_t[:, O_P2:O_T2].bitcast(bf16)
            t2_v = td_t[:, O_T2:W_ALL].bitcast(bf16)

            st_t = pool.tile([128, 8], f32, tag="st")
            stats = st_t[:]

            # ---- ACT: Ln for the stop term ----
            lnp_t = pool.tile([128, MG], f16, tag="lnp")
            nc.scalar.activation(lnp_t[:], stop_v, Act.Ln)

            # ---- DVE stop front + dc term (phase S2 data only) ----
            tl_t = pool.tile([128, MG], f16, tag="tl")
            nc.vector.scalar_tensor_tensor(
                tl_t[:], iota_v, 1.0, mask_v, op0=Alu.bypass, op1=Alu.mult)
            mxp_t = pool.tile([128, 1], f16, tag="mxp")
            nc.vector.tensor_reduce(mxp_t[:], tl_t[:], axis=Ax.X, op=Alu.max)
            nc.vector.tensor_reduce(stats[:, 3:4], mask_v, axis=Ax.X,
                                    op=Alu.add)
            nc.vector.tensor_copy(stats[:, 4:5], lpre_v)
            asum_t = pool.tile([128, ALN_HALF], bf16, tag="asum")
            dcs_t = pool.tile([128, 1], f32, tag="dcs")
            nc.vector.scalar_tensor_tensor(
                asum_t[:], aln_v[:, 0:ALN_HALF], 1.0, aln_v[:, ALN_HALF:],
                op0=Alu.bypass, op1=Alu.add, accum_out=dcs_t[:])
            nc.vector.scalar_tensor_tensor(
                stats[:, 0:1], lenf_v, float(T), dcs_t[:],
                op0=Alu.is_le, op1=Alu.mult)
            eq_t = pool.tile([128, MG], f16, tag="eq")
            cp_t = pool.tile([128, 1], f16, tag="cp")
            nc.vector.scalar_tensor_tensor(
                eq_t[:], tl_t[:], mxp_t[:, 0:1], lnp_t[:],
                op0=Alu.is_equal, op1=Alu.mult, accum_out=cp_t[:])

            # ---- stop tail: PE transposes, per-b max, select ----
            psA = ppool.tile([1, 128], f32, tag="psA")
            nc.tensor.matmul(psA[:], lhsT=mxp_t[:], rhs=id_v,
                             start=True, stop=True)
            psB = ppool.tile([1, 128], f32, tag="psB")
            nc.tensor.matmul(psB[:], lhsT=cp_t[:], rhs=id_v,
                             start=True, stop=True)
            sbA_t = pool.tile([1, 128], f32, tag="sbA")
            nc.vector.tensor_copy(sbA_t[:], psA[:])
            mb_t = pool.tile([1, 2], f32, tag="mb")
            nc.vector.tensor_reduce(
                mb_t[:], sbA_t[:].rearrange("p (b g) -> p b g", g=64),
                axis=Ax.X, op=Alu.max)
            e0_t = pool.tile([1, 64], f32, tag="e0")
            nc.vector.scalar_tensor_tensor(
                e0_t[:], sbA_t[0:1, 0:64], mb_t[:, 0:1], psB[0:1, 0:64],
                op0=Alu.is_equal, op1=Alu.mult, accum_out=stats[0:1, 2:3])
            e1_t = pool.tile([1, 64], f32, tag="e1")
            nc.vector.scalar_tensor_tensor(
                e1_t[:], sbA_t[0:1, 64:128], mb_t[:, 1:2], psB[0:1, 64:128],
                op0=Alu.is_equal, op1=Alu.mult, accum_out=stats[0:1, 5:6])

            # ---- mel term: halves pipelined behind phases S1a / S1b ----
            d_t = pool.tile([128, MEL_F], bf16, tag="d")
            nc.vector.tensor_sub(d_t[:, 0:MH], p1_v, t1_v)
            nc.vector.tensor_reduce(stats[:, 1:2], d_t[:, 0:MH], axis=Ax.X,
                                    op=Alu.add, apply_absolute_value=True)
            nc.vector.tensor_sub(d_t[:, MH:MEL_F], p2_v, t2_v)
            nc.vector.tensor_reduce(stats[:, 6:7], d_t[:, MH:MEL_F],
                                    axis=Ax.X, op=Alu.add,
                                    apply_absolute_value=True)

            # ---- stats go out raw; the host folds the 128 partitions ----
            nc.sync.dma_start(out, st_t[:])

    nc.compile()
    return nc


def _get_nc():
    if "nc" not in _CACHE:
        _CACHE["nc"] = _build_bass()
    return _CACHE["nc"]


def make_in_maps(lengths, mask, stop_pred, mels_pred, mels_target, alignments):
    """Shard full inputs into the 8 per-core input dicts."""
    lengths = np.ascontiguousarray(lengths, dtype=np.int32)
    maskf = np.ascontiguousarray(mask).astype(np.float32)
    stop_pred = np.ascontiguousarray(stop_pred, dtype=np.float32)
    mels_pred = np.ascontiguousarray(mels_pred, dtype=np.float32)
    mels_target = np.ascontiguousarray(mels_target, dtype=np.float32)
    alignments = np.ascontiguousarray(alignments, dtype=np.float32)

    bf = ml_dtypes.bfloat16
    f8 = ml_dtypes.float8_e4m3
    band = _band_bool()  # [S, TC]
    el = alignments[:, :, :, :TC][:, :, band]  # [N, B*H, 2975]

    def split13(row, pad):
        o = np.full((64 * MG,), pad, row.dtype)
        o[:T] = row
        return o.reshape(64, MG)

    iota13 = np.concatenate([split13(np.arange(1, T + 1, dtype=np.float16),
                                     np.float16(0))] * 2)  # [128, 13]
    ident = np.eye(128, dtype=np.float16)

    def pad_rows(x2d):
        padded = np.zeros((MEL_PAD_ROWS, NMEL), x2d.dtype)
        padded[:MEL_ROWS] = x2d
        return padded.reshape(128, MEL_F)

    in_maps = []
    for c in range(NCORES):
        bs = slice(2 * c, 2 * c + 2)
        mp = pad_rows((mels_pred[bs] * maskf[bs][..., None])
                      .reshape(MEL_ROWS, NMEL).astype(bf))
        mt = pad_rows(mels_target[bs].reshape(MEL_ROWS, NMEL).astype(bf))

        aln = np.zeros((8, 16 * ALN_PER_PART), f8)
        core_el = el[:, 8 * c:8 * c + 8]          # [3, 8, 2975]
        aln[:, :N * ALN_PER_PLANE] = \
            core_el.transpose(1, 0, 2).reshape(8, N * ALN_PER_PLANE).astype(f8)
        aln = aln.reshape(128, ALN_PER_PART)

        dAll = np.zeros((128, W_ALL), np.uint8)
        dAll[:, O_ID:O_ALN] = ident.view(np.uint8)
        dAll[:, O_ALN:O_STOP] = aln.view(np.uint8)
        st13 = np.concatenate(
            [split13(stop_pred[2 * c].astype(np.float16), np.float16(1.0)),
             split13(stop_pred[2 * c + 1].astype(np.float16), np.float16(1.0))])
        mk13 = np.concatenate(
            [split13(maskf[2 * c].astype(np.float16), np.float16(0)),
             split13(maskf[2 * c + 1].astype(np.float16), np.float16(0))])
        dAll[:, O_STOP:O_MASK] = st13.view(np.uint8)
        dAll[:, O_MASK:O_IOTA] = mk13.view(np.uint8)
        dAll[:, O_IOTA:O_IOTA + 2 * MG] = iota13.view(np.uint8)
        lenf = np.repeat(lengths[bs].astype(np.float32), 64)  # [128]
        dAll[:, O_LEN:O_LEN + 4] = lenf[:, None].view(np.uint8)
        lpre = np.zeros((128, 1), np.float32)
        lpre[0:2, 0] = lengths[bs]
        dAll[:, O_LPRE:O_LPRE + 4] = lpre.view(np.uint8)
        dAll[:, O_P1:O_T1] = mp[:, 0:MH].view(np.uint8)
        dAll[:, O_T1:O_P2] = np.ascontiguousarray(mt[:, 0:MH]).view(np.uint8)
        dAll[:, O_P2:O_T2] = np.ascontiguousarray(mp[:, MH:]).view(np.uint8)
        dAll[:, O_T2:W_ALL] = np.ascontiguousarray(mt[:, MH:]).view(np.uint8)

        in_maps.append({"dAll": dAll})
    return in_maps


def combine_partials(partials):
    """partials: list of 8 arrays [128,8] -> final scalar (0-d f32 ndarray).

    Cols 2/5 (stop selection) are only written on partition 0; the other
    partitions hold uninitialized SBUF and are ignored. Col 7 is unused.
    """
    ps = np.stack([np.asarray(p, dtype=np.float64).reshape(128, 8)
                   for p in partials])
    tot = ps.sum(axis=(0, 1))
    p0 = ps[:, 0, :].sum(axis=0)
    dc_w, mask_cnt, len_sum = tot[0], tot[3], tot[4]
    melA = tot[1] + tot[6]
    sel_lnp = p0[2] + p0[5]
    mel_loss = melA / float(B * T * NMEL)
    stop_loss = -5.0 * sel_lnp / mask_cnt
    dc = dc_w / (H * len_sum * N)
    return np.array(np.float32(mel_loss + stop_loss - 1e-4 * dc))


def kernel(lengths, mask, stop_pred, mels_pred, mels_target, alignments):
    from concourse.bass_utils import run_bass_kernel_spmd

    nc = _get_nc()
    in_maps = make_in_maps(lengths, np.asarray(mask), stop_pred,
                           mels_pred, mels_target, alignments)
    res = run_bass_kernel_spmd(nc, in_maps, list(range(NCORES)))
    return combine_partials([r["out"] for r in res.results])
